# revision 23
# baseline (speedup 1.0000x reference)
"""Trainium2 Bass kernel for nn_Decoder (2-layer LSTM decoder + vocab head).

Computation (matches reference.py):
  embeds = emb[sentence]                      [B, T, E]
  x = concat(features, embeds[:, :-1])        [B, T, E]
  h0 = LSTM0(x), h1 = LSTM1(h0)               [B, T, H]
  out = (h1 @ fc_W.T + fc_b).transpose(0,2,1) [B, V, T]

Sharding (8 NeuronCores, SPMD, no collectives): data-parallel over batch.
Each core runs the full 2-layer LSTM for its 8 sequences and the full
32000-row vocab head for those sequences.  Compared to replicating the
LSTM and vocab-splitting the head, this cuts the per-core input
projections 8x at the cost of streaming the fc weight matrix per core
(e3m4, 16 MB), largely prefetched during the LSTM phase.

Layout: k-space (contraction dim on partitions), tokens t-major
(tok = t*8 + b).  Gates are reordered [i, f, o, g] on the host so the
three sigmoid gates are one contiguous ScalarE activation.  The per-core
output is [128, 250, 256] (p, vtile, tok); host maps v = vt*128 + p and
transposes to [B, V, T].

Schedule:
  xp0 half 0 -> rec0 starts; xp0 half 1 fills the first chain gap
  rec0(t), t in 0..31; after rec0 step 8q+7: xp1 quarter q (DVE move)
  rec1(t) lags by 8; during the rec1-only tail, fc chunks 0..7 token-half
  0 interleave as tensor filler; the rest of fc runs after.
fc weights are e3m4 (x256 scale, mixed-dtype matmul against bf16 h1);
the 1/256 compensation folds into the bias-add that reads PSUM.

Environment note: this walrus build rejects >1 embedded sync wait per
instruction; _split_waits_json() rewrites the serialized BIR, hoisting
excess waits onto same-engine NoOp carriers (identical semantics).
"""

import numpy as np
import ml_dtypes

import orjson
import concourse.tile as tile

_MAXW = 1


def _split_waits_json(b: bytes) -> bytes:
    d = orjson.loads(b)
    for f in d["functions"]:
        for blk in f["blocks"]:
            out = []
            for inst in blk["instructions"]:
                si = inst.get("sync_info")
                if si:
                    w = si.get("on_wait") or []
                    if len(w) > _MAXW:
                        for i, wt in enumerate(w[:-_MAXW]):
                            out.append(
                                {
                                    "debug": inst.get("debug", 0),
                                    "engine": inst["engine"],
                                    "ins": [],
                                    "outs": [],
                                    "name": f"{inst['name']}-hw{i}",
                                    "opcode": "NoOp",
                                    "sync_info": {"on_update": [], "on_wait": [wt]},
                                }
                            )
                        si["on_wait"] = w[-_MAXW:]
                out.append(inst)
            blk["instructions"] = out
    return orjson.dumps(d)


def _patch_serialization(nc):
    orig = nc.to_json_bytes
    nc.to_json_bytes = lambda: _split_waits_json(orig())
    return nc


import concourse.bass as bass
import concourse.mybir as mybir
from concourse.bass import ts, ds
from concourse.bass_utils import run_bass_kernel_spmd

F32 = mybir.dt.float32
BF16 = mybir.dt.bfloat16
E3M4 = mybir.dt.float8e3
AF = mybir.ActivationFunctionType
ALU = mybir.AluOpType
BF16_NP = ml_dtypes.bfloat16
E3M4_NP = ml_dtypes.float8_e3m4

E, H, V, B, T = 512, 512, 32000, 64, 32
G = 4 * H                    # 2048 gate rows per layer
KC = 4                       # 512 = 4 k-chunks of 128
NCORES = 8
BL = B // NCORES             # 8 sequences per core
NTOK = BL * T                # 256 tokens per core
VT = V // 128                # 250 vocab tiles
LAG = 8                      # rec1 runs LAG steps behind rec0
FCCH = 16                    # fc weight chunk: 16 vocab tiles
FCG = 4                      # vocab tiles staged per output DMA
FCSCALE = 256.0              # e3m4 weight scale (compensated at PSUM read)
NCHUNKS = (VT + FCCH - 1) // FCCH


def _build_nc():
    nc = bass.Bass()

    xT_d = nc.dram_tensor("xT", [128, KC, NTOK], BF16, kind="ExternalInput")
    wih0_d = nc.dram_tensor("wih0T", [128, KC, G], BF16, kind="ExternalInput")
    whh0_d = nc.dram_tensor("whh0T", [128, KC, G], BF16, kind="ExternalInput")
    wih1_d = nc.dram_tensor("wih1T", [128, KC, G], BF16, kind="ExternalInput")
    whh1_d = nc.dram_tensor("whh1T", [128, KC, G], BF16, kind="ExternalInput")
    b0_d = nc.dram_tensor("b0", [128, 16], F32, kind="ExternalInput")
    b1_d = nc.dram_tensor("b1", [128, 16], F32, kind="ExternalInput")
    ident_d = nc.dram_tensor("ident", [128, 128], BF16, kind="ExternalInput")
    fcw_d = nc.dram_tensor("fcwT", [128, KC, V], E3M4, kind="ExternalInput")
    fcb_d = nc.dram_tensor("fcb", [128, VT], F32, kind="ExternalInput")
    out_d = nc.dram_tensor("out", [128, VT, NTOK], F32, kind="ExternalOutput")

    with tile.TileContext(nc) as tc:
        with (
            tc.tile_pool(name="consts", bufs=1) as consts,
            tc.tile_pool(name="state", bufs=1) as state,
            tc.tile_pool(name="fcw", bufs=12) as fcwp,
            tc.tile_pool(name="fcstage", bufs=8) as fcstage,
            tc.tile_pool(name="ps_gates", bufs=4, space="PSUM") as ps_gates,
            tc.tile_pool(name="ps_wide", bufs=4, space="PSUM") as ps_wide,
        ):
            b0_sb = consts.tile([128, 16], F32, tag="b0")
            b1_sb = consts.tile([128, 16], F32, tag="b1")
            fcb_sb = consts.tile([128, VT], F32, tag="fcb")
            ident = consts.tile([128, 128], BF16, tag="ident")

            hist0 = consts.tile([128, KC, T, BL], BF16, tag="hist0")
            hist1 = consts.tile([128, KC, T, BL], BF16, tag="hist1")
            xp0r = consts.tile([128, 16, NTOK], BF16, tag="xp0r")
            xp1r = consts.tile([128, 16, NTOK], BF16, tag="xp1r")

            st = []
            for l in range(2):
                st.append(
                    dict(
                        cT=state.tile([128, KC, BL], F32, name=f"cT{l}"),
                        gates=state.tile([128, 16, BL], F32, name=f"gates{l}"),
                        tmp1=state.tile([128, KC, BL], F32, name=f"tmp1{l}"),
                        tmp2=state.tile([128, KC, BL], F32, name=f"tmp2{l}"),
                        tanh_c=state.tile([128, KC, BL], F32, name=f"tanhc{l}"),
                    )
                )

            def xp_block(w_sb, rhs, bias_sb, ring, n0, ncols, vec):
                """ring[:, g, n0:n0+ncols] = W.T @ rhs(kc) + bias, all 16 g."""
                for g in range(16):
                    ps = ps_wide.tile([128, ncols], F32, tag="psw")
                    for kc in range(KC):
                        nc.tensor.matmul(
                            ps,
                            w_sb[:, kc, ts(g, 128)],
                            rhs(kc),
                            start=(kc == 0),
                            stop=(kc == KC - 1),
                        )
                    if vec:
                        nc.vector.tensor_scalar_add(
                            ring[:, g, ds(n0, ncols)], ps, bias_sb[:, g : g + 1]
                        )
                    else:
                        nc.scalar.activation(
                            out=ring[:, g, ds(n0, ncols)], in_=ps, func=AF.Identity,
                            bias=bias_sb[:, g : g + 1], scale=1.0,
                        )

            SLK = [0]                  # step-layer counter for chain pacing
            PACE = 0.0022              # ms per step-layer (both layers live)
            SOLO = 0.0036              # ms per step-layer while rec0 is alone
            NSOLO = 8
            RBASE = 0.008              # ms offset of the first rec block

            def slk_base(k):
                if k < NSOLO:
                    return RBASE + k * SOLO
                return RBASE + NSOLO * SOLO + (k - NSOLO) * PACE

            def rec_step(l, t, whh_sb, ring, hist):
                # gate order [i(0:4) f(4:8) o(8:12) g(12:16)].  The identity
                # fold is split so the sigmoid ACT (slabs 0:12) unblocks
                # before the tail of the matmul group finishes.
                # The scheduler's cost model treats LDWEIGHTS as free, so it
                # underestimates rec blocks and orders chain tails after the
                # next block's gate ACTs (head-of-line blocking).  Floors on
                # the chain ops pin the intended order; they sit below real
                # hardware pace, so they cost nothing at runtime.
                base = slk_base(SLK[0])
                SLK[0] += 1
                s = st[l]
                ps = ps_gates.tile([128, 16, BL], F32, tag="psg")
                def rec_mm(j):
                    for kc in range(KC):
                        nc.tensor.matmul(
                            ps[:, j, :],
                            whh_sb[:, kc, ts(j, 128)],
                            hist[:, kc, t - 1, :],
                            start=(j == 0 and kc == 0),
                            stop=False,
                            skip_group_check=True,
                        )
                if t > 0:
                    for j in range(12):
                        rec_mm(j)
                nc.tensor.matmul(
                    ps[:, 0:12, :],
                    ident,
                    ring[:, 0:12, ds(t * BL, BL)],
                    start=(t == 0),
                    stop=False,
                    skip_group_check=True,
                )
                if t > 0:
                    for j in range(12, 16):
                        rec_mm(j)
                nc.tensor.matmul(
                    ps[:, 12:16, :],
                    ident,
                    ring[:, 12:16, ds(t * BL, BL)],
                    start=False,
                    stop=True,
                    skip_group_check=True,
                )
                g = s["gates"]
                def chain_op(off, fn):
                    with tc.tile_wait_until(base + off), tc.high_priority(
                        offset=500000
                    ):
                        fn()
                chain_op(0.00220, lambda: nc.scalar.activation(
                    g[:, 0:12, :], ps[:, 0:12, :], func=AF.Sigmoid))
                chain_op(0.00230, lambda: nc.scalar.activation(
                    g[:, 12:16, :], ps[:, 12:16, :], func=AF.Tanh))
                if t == 0:
                    chain_op(0.00240, lambda: nc.vector.tensor_mul(
                        s["cT"], g[:, 0:4, :], g[:, 12:16, :]))
                else:
                    chain_op(0.00235, lambda: nc.vector.tensor_mul(
                        s["tmp2"], g[:, 4:8, :], s["cT"]))
                    chain_op(0.00240, lambda: nc.vector.tensor_mul(
                        s["tmp1"], g[:, 0:4, :], g[:, 12:16, :]))
                    chain_op(0.00245, lambda: nc.vector.tensor_add(
                        s["cT"], s["tmp1"], s["tmp2"]))
                chain_op(0.00255, lambda: nc.scalar.activation(
                    s["tanh_c"], s["cT"], func=AF.Tanh))
                chain_op(0.00265, lambda: nc.vector.tensor_mul(
                    hist[:, :, t, :], g[:, 8:12, :], s["tanh_c"]))

            # ---------------- fc helpers -----------------------------------
            fcw_tiles = {}
            fc_ct = [0]

            def fc_chunk_dma(ch, eng=None):
                v0 = ch * FCCH
                nvt = min(FCCH, VT - v0)
                fcw_sb = fcwp.tile(
                    [128, KC, FCCH * 128], E3M4, tag="fcw", name=f"fcw{ch}"
                )
                fcw_tiles[ch] = fcw_sb
                (eng or nc.gpsimd).dma_start(
                    out=fcw_sb[:, :, ds(0, nvt * 128)],
                    in_=fcw_d[:, :, ds(v0 * 128, nvt * 128)],
                )

            def fc_half(ch, half, alt_psum=False):
                v0 = ch * FCCH
                nvt = min(FCCH, VT - v0)
                fcw_sb = fcw_tiles[ch]
                if ch >= NCHUNKS - 2:
                    dma_engines = [nc.scalar, nc.sync]
                else:
                    dma_engines = [nc.sync, nc.gpsimd]
                for g0 in range(0, nvt, FCG):
                    ng = min(FCG, nvt - g0)
                    ot = fcstage.tile([128, FCG, 128], F32, tag="ot")
                    for j in range(ng):
                        vt = v0 + g0 + j
                        if alt_psum and (fc_ct[0] // 2) % 2 == 0:
                            ps = ps_gates.tile([128, 128], F32, tag="psg")
                        else:
                            ps = ps_wide.tile([128, 128], F32, tag="psw")
                        for kc in range(KC):
                            nc.tensor.matmul(
                                ps,
                                fcw_sb[:, kc, ts(g0 + j, 128)],
                                hist1[:, kc, ts(half, 16), :],
                                start=(kc == 0),
                                stop=(kc == KC - 1),
                            )
                        r = fc_ct[0] % 2
                        if r == 0:
                            nc.scalar.activation(
                                out=ot[:, j, :], in_=ps, func=AF.Identity,
                                bias=fcb_sb[:, vt : vt + 1], scale=1.0 / FCSCALE,
                            )
                        else:
                            nc.vector.tensor_scalar(
                                ot[:, j, :], ps, 1.0 / FCSCALE,
                                fcb_sb[:, vt : vt + 1], op0=ALU.mult, op1=ALU.add,
                            )
                        fc_ct[0] += 1
                    eng = dma_engines[(g0 // FCG) % 2]
                    eng.dma_start(
                        out=out_d[:, ds(v0 + g0, ng), ts(half, 128)],
                        in_=ot[:, ds(0, ng), :],
                    )

            with tc.tile_pool(name="wpool", bufs=1) as wpool:
                nc.scalar.dma_start(out=b0_sb, in_=b0_d[:])
                nc.scalar.dma_start(out=b1_sb, in_=b1_d[:])
                nc.scalar.dma_start(out=ident, in_=ident_d[:])
                nc.scalar.dma_start(out=fcb_sb, in_=fcb_d[:])

                xT_sb = wpool.tile([128, KC, NTOK], BF16, tag="xT")
                wih0_sb = wpool.tile([128, KC, G], BF16, tag="wih0")
                whh0_sb = wpool.tile([128, KC, G], BF16, tag="whh0")
                wih1_sb = wpool.tile([128, KC, G], BF16, tag="wih1")
                whh1_sb = wpool.tile([128, KC, G], BF16, tag="whh1")
                nc.sync.dma_start(
                    out=wih0_sb[:, :, ds(0, 1024)], in_=wih0_d[:, :, ds(0, 1024)]
                )
                nc.sync.dma_start(out=xT_sb, in_=xT_d[:])
                nc.sync.dma_start(
                    out=wih0_sb[:, :, ds(1024, 1024)],
                    in_=wih0_d[:, :, ds(1024, 1024)],
                )
                for piece in range(2):
                    nc.sync.dma_start(
                        out=whh0_sb[:, :, ts(piece, 1024)],
                        in_=whh0_d[:, :, ts(piece, 1024)],
                    )
                nc.sync.dma_start(out=wih1_sb, in_=wih1_d[:])
                nc.sync.dma_start(out=whh1_sb, in_=whh1_d[:])

                rec0 = dict(whh_sb=whh0_sb, ring=xp0r, hist=hist0)
                rec1 = dict(whh_sb=whh1_sb, ring=xp1r, hist=hist1)

                # prefetch the first 12 fc weight chunks BEHIND the LSTM
                # weights on the same FIFO queue: weights land first at full
                # DMA rate, then fcw streams during the DMA-idle LSTM phase
                for ch in range(12):
                    fc_chunk_dma(ch, eng=nc.sync)
                # first-layer input projection in token quarters: quarter 0
                # covers rec0 steps 0-7; the rest fill the rec0-solo gaps
                def xp0_quarter(q):
                    xp_block(
                        wih0_sb, lambda kc: xT_sb[:, kc, ds(q * 64, 64)],
                        b0_sb, xp0r, q * 64, 64, vec=True,
                    )

                xp0_quarter(0)
                for t in range(T):
                    rec_step(0, t, **rec0)
                    if t in (0, 2, 4):
                        xp0_quarter(t // 2 + 1)
                    if t % LAG == LAG - 1:
                        q = t // LAG
                        xp_block(
                            wih1_sb,
                            lambda kc: hist0[:, kc, ts(q, LAG), :],
                            b1_sb,
                            xp1r,
                            q * LAG * BL,
                            LAG * BL,
                            vec=True,
                        )
                    if t >= LAG:
                        rec_step(1, t - LAG, **rec1)
                # rec1 tail: fc h0 of the prefetched chunks fills the gaps
                for i, s_ in enumerate(range(T - LAG, T)):
                    rec_step(1, s_, **rec1)
                    fc_half(i, 0)

            # ================= fc remainder ================================
            # h1 of the resident chunks frees their buffers; stream the rest
            # h1 of chunks 0..3 first: their buffers feed the late chunks
            # (12..15), which then stream while the middle chunks compute
            for ch in range(4):
                fc_half(ch, 1, alt_psum=True)
                fc_chunk_dma(ch + 12, eng=nc.sync)
            for ch in range(LAG, 12):
                fc_half(ch, 0, alt_psum=True)
            for ch in range(4, 12):
                fc_half(ch, 1, alt_psum=True)
            for ch in range(12, NCHUNKS):
                fc_half(ch, 0, alt_psum=True)
                fc_half(ch, 1, alt_psum=True)
    return _patch_serialization(nc)


def _to_k128(W, dtype):
    """W [out_dim, K] -> [128, K//128, out_dim] with result[p,kc,g]=W[g,kc*128+p]."""
    K = W.shape[1]
    return np.ascontiguousarray(
        W.T.reshape(K // 128, 128, -1).transpose(1, 0, 2)
    ).astype(dtype)


# PyTorch gate order [i f g o] -> device order [i f o g]
_PERM = np.concatenate(
    [np.arange(0, 1024), np.arange(1536, 2048), np.arange(1024, 1536)]
)

_NC_CACHE = None
RUN_KWARGS = {}
LAST_RESULT = None


def kernel(
    sentence,
    features,
    lengths,
    emb,
    W_ih0,
    W_hh0,
    b_ih0,
    b_hh0,
    W_ih1,
    W_hh1,
    b_ih1,
    b_hh1,
    fc_W,
    fc_b,
):
    global _NC_CACHE, LAST_RESULT
    sentence = np.asarray(sentence).astype(np.int64)
    features = np.asarray(features, dtype=np.float32)
    emb = np.asarray(emb, dtype=np.float32)

    # embedding gather + teacher forcing shift (host; pure data movement)
    embeds = emb[sentence[:, : T - 1]]                      # [B, T-1, E]
    x = np.concatenate([features[:, None, :], embeds], axis=1)  # [B, T, E]

    wih0 = _to_k128(np.asarray(W_ih0, np.float32)[_PERM], BF16_NP)
    whh0 = _to_k128(np.asarray(W_hh0, np.float32)[_PERM], BF16_NP)
    wih1 = _to_k128(np.asarray(W_ih1, np.float32)[_PERM], BF16_NP)
    whh1 = _to_k128(np.asarray(W_hh1, np.float32)[_PERM], BF16_NP)
    b0 = np.ascontiguousarray(
        (np.asarray(b_ih0, np.float32) + np.asarray(b_hh0, np.float32))[_PERM]
        .reshape(16, 128)
        .T
    )
    b1 = np.ascontiguousarray(
        (np.asarray(b_ih1, np.float32) + np.asarray(b_hh1, np.float32))[_PERM]
        .reshape(16, 128)
        .T
    )
    fcw = _to_k128(
        np.asarray(fc_W, np.float32) * FCSCALE, E3M4_NP
    )                                                       # [128, KC, V]
    fcb = np.ascontiguousarray(
        np.asarray(fc_b, np.float32).reshape(VT, 128).T
    )

    common = {
        "wih0T": wih0,
        "whh0T": whh0,
        "wih1T": wih1,
        "whh1T": whh1,
        "b0": b0,
        "b1": b1,
        "ident": np.eye(128, dtype=BF16_NP),
        "fcwT": fcw,
        "fcb": fcb,
    }
    in_maps = []
    for c in range(NCORES):
        xc = x[c * BL : (c + 1) * BL]                       # [BL, T, E]
        # token-major [k, tok] with tok = t*BL + b
        xT = np.ascontiguousarray(xc.transpose(2, 1, 0).reshape(E, NTOK))
        xT_p = np.ascontiguousarray(
            xT.reshape(KC, 128, NTOK).transpose(1, 0, 2)
        ).astype(BF16_NP)
        in_maps.append({**common, "xT": xT_p})

    if _NC_CACHE is None:
        _NC_CACHE = _build_nc()

    res = run_bass_kernel_spmd(
        _NC_CACHE, in_maps, core_ids=list(range(NCORES)), **RUN_KWARGS
    )
    LAST_RESULT = res
    # per-core out: [128, VT, NTOK] (v = vt*128 + p) -> [V, T, BL] -> [BL, V, T]
    full = np.concatenate(
        [
            res.results[c]["out"]
            .transpose(1, 0, 2)
            .reshape(V, T, BL)
            .transpose(2, 0, 1)
            for c in range(NCORES)
        ],
        axis=0,
    )  # [B, V, T]
    return np.ascontiguousarray(full)


# revision 25
# speedup vs baseline: 1.0111x; 1.0111x over previous
"""Trainium2 Bass kernel for nn_Decoder (2-layer LSTM decoder + vocab head).

Computation (matches reference.py):
  embeds = emb[sentence]                      [B, T, E]
  x = concat(features, embeds[:, :-1])        [B, T, E]
  h0 = LSTM0(x), h1 = LSTM1(h0)               [B, T, H]
  out = (h1 @ fc_W.T + fc_b).transpose(0,2,1) [B, V, T]

Sharding (8 NeuronCores, SPMD, no collectives): data-parallel over batch.
Each core runs the full 2-layer LSTM for its 8 sequences and the full
32000-row vocab head for those sequences.  Compared to replicating the
LSTM and vocab-splitting the head, this cuts the per-core input
projections 8x at the cost of streaming the fc weight matrix per core
(e3m4, 16 MB), largely prefetched during the LSTM phase.

Layout: k-space (contraction dim on partitions), tokens t-major
(tok = t*8 + b).  Gates are reordered [i, f, o, g] on the host so the
three sigmoid gates are one contiguous ScalarE activation.  The per-core
output is [128, 250, 256] (p, vtile, tok); host maps v = vt*128 + p and
transposes to [B, V, T].

Schedule:
  xp0 half 0 -> rec0 starts; xp0 half 1 fills the first chain gap
  rec0(t), t in 0..31; after rec0 step 8q+7: xp1 quarter q (DVE move)
  rec1(t) lags by 8; during the rec1-only tail, fc chunks 0..7 token-half
  0 interleave as tensor filler; the rest of fc runs after.
fc weights are e3m4 (x256 scale, mixed-dtype matmul against bf16 h1);
the 1/256 compensation folds into the bias-add that reads PSUM.

Environment note: this walrus build rejects >1 embedded sync wait per
instruction; _split_waits_json() rewrites the serialized BIR, hoisting
excess waits onto same-engine NoOp carriers (identical semantics).
"""

import numpy as np
import ml_dtypes

import orjson
import concourse.tile as tile

_MAXW = 1


def _split_waits_json(b: bytes) -> bytes:
    d = orjson.loads(b)
    for f in d["functions"]:
        for blk in f["blocks"]:
            out = []
            for inst in blk["instructions"]:
                si = inst.get("sync_info")
                if si:
                    w = si.get("on_wait") or []
                    if len(w) > _MAXW:
                        for i, wt in enumerate(w[:-_MAXW]):
                            out.append(
                                {
                                    "debug": inst.get("debug", 0),
                                    "engine": inst["engine"],
                                    "ins": [],
                                    "outs": [],
                                    "name": f"{inst['name']}-hw{i}",
                                    "opcode": "NoOp",
                                    "sync_info": {"on_update": [], "on_wait": [wt]},
                                }
                            )
                        si["on_wait"] = w[-_MAXW:]
                out.append(inst)
            blk["instructions"] = out
    return orjson.dumps(d)


def _patch_serialization(nc):
    orig = nc.to_json_bytes
    nc.to_json_bytes = lambda: _split_waits_json(orig())
    return nc


import concourse.bass as bass
import concourse.mybir as mybir
from concourse.bass import ts, ds
from concourse.bass_utils import run_bass_kernel_spmd

F32 = mybir.dt.float32
BF16 = mybir.dt.bfloat16
E3M4 = mybir.dt.float8e3
AF = mybir.ActivationFunctionType
ALU = mybir.AluOpType
BF16_NP = ml_dtypes.bfloat16
E3M4_NP = ml_dtypes.float8_e3m4

E, H, V, B, T = 512, 512, 32000, 64, 32
G = 4 * H                    # 2048 gate rows per layer
KC = 4                       # 512 = 4 k-chunks of 128
NCORES = 8
BL = B // NCORES             # 8 sequences per core
NTOK = BL * T                # 256 tokens per core
VT = V // 128                # 250 vocab tiles
LAG = 8                      # rec1 runs LAG steps behind rec0
FCCH = 16                    # fc weight chunk: 16 vocab tiles
FCG = 4                      # vocab tiles staged per output DMA
FCSCALE = 256.0              # e3m4 weight scale (compensated at PSUM read)
NCHUNKS = (VT + FCCH - 1) // FCCH


def _build_nc():
    nc = bass.Bass()

    xT_d = nc.dram_tensor("xT", [128, KC, NTOK], BF16, kind="ExternalInput")
    wih0_d = nc.dram_tensor("wih0T", [128, KC, G], BF16, kind="ExternalInput")
    whh0_d = nc.dram_tensor("whh0T", [128, KC, G], BF16, kind="ExternalInput")
    wih1_d = nc.dram_tensor("wih1T", [128, KC, G], BF16, kind="ExternalInput")
    whh1_d = nc.dram_tensor("whh1T", [128, KC, G], BF16, kind="ExternalInput")
    b0_d = nc.dram_tensor("b0", [128, 16], F32, kind="ExternalInput")
    b1_d = nc.dram_tensor("b1", [128, 16], F32, kind="ExternalInput")
    ident_d = nc.dram_tensor("ident", [128, 128], BF16, kind="ExternalInput")
    fcw_d = nc.dram_tensor("fcwT", [128, KC, V], E3M4, kind="ExternalInput")
    fcb_d = nc.dram_tensor("fcb", [128, VT], F32, kind="ExternalInput")
    out_d = nc.dram_tensor("out", [128, VT, NTOK], F32, kind="ExternalOutput")

    with tile.TileContext(nc) as tc:
        with (
            tc.tile_pool(name="consts", bufs=1) as consts,
            tc.tile_pool(name="state", bufs=1) as state,
            tc.tile_pool(name="fcw", bufs=12) as fcwp,
            tc.tile_pool(name="fcstage", bufs=8) as fcstage,
            tc.tile_pool(name="ps_gates", bufs=4, space="PSUM") as ps_gates,
            tc.tile_pool(name="ps_wide", bufs=4, space="PSUM") as ps_wide,
        ):
            b0_sb = consts.tile([128, 16], F32, tag="b0")
            b1_sb = consts.tile([128, 16], F32, tag="b1")
            fcb_sb = consts.tile([128, VT], F32, tag="fcb")
            ident = consts.tile([128, 128], BF16, tag="ident")

            hist0 = consts.tile([128, KC, T, BL], BF16, tag="hist0")
            hist1 = consts.tile([128, KC, T, BL], BF16, tag="hist1")
            xp0r = consts.tile([128, 16, NTOK], BF16, tag="xp0r")
            xp1r = consts.tile([128, 16, NTOK], BF16, tag="xp1r")

            st = []
            for l in range(2):
                st.append(
                    dict(
                        cT=state.tile([128, KC, BL], F32, name=f"cT{l}"),
                        gates=state.tile([128, 16, BL], F32, name=f"gates{l}"),
                        tmp1=state.tile([128, KC, BL], F32, name=f"tmp1{l}"),
                        tmp2=state.tile([128, KC, BL], F32, name=f"tmp2{l}"),
                        tanh_c=state.tile([128, KC, BL], F32, name=f"tanhc{l}"),
                    )
                )

            def xp_block(w_sb, rhs, bias_sb, ring, n0, ncols, vec):
                """ring[:, g, n0:n0+ncols] = W.T @ rhs(kc) + bias, all 16 g."""
                for g in range(16):
                    ps = ps_wide.tile([128, ncols], F32, tag="psw")
                    for kc in range(KC):
                        nc.tensor.matmul(
                            ps,
                            w_sb[:, kc, ts(g, 128)],
                            rhs(kc),
                            start=(kc == 0),
                            stop=(kc == KC - 1),
                        )
                    if vec and g % 2 == 0:
                        nc.vector.tensor_scalar_add(
                            ring[:, g, ds(n0, ncols)], ps, bias_sb[:, g : g + 1]
                        )
                    else:
                        nc.scalar.activation(
                            out=ring[:, g, ds(n0, ncols)], in_=ps, func=AF.Identity,
                            bias=bias_sb[:, g : g + 1], scale=1.0,
                        )

            SLK = [0]                  # step-layer counter for chain pacing
            PACE = 0.0022              # ms per step-layer (both layers live)
            SOLO = 0.0036              # ms per step-layer while rec0 is alone
            NSOLO = 8
            RBASE = 0.008              # ms offset of the first rec block

            def slk_base(k):
                if k < NSOLO:
                    return RBASE + k * SOLO
                return RBASE + NSOLO * SOLO + (k - NSOLO) * PACE

            def rec_step(l, t, whh_sb, ring, hist):
                # gate order [i(0:4) f(4:8) o(8:12) g(12:16)].  The identity
                # fold is split so the sigmoid ACT (slabs 0:12) unblocks
                # before the tail of the matmul group finishes.
                # The scheduler's cost model treats LDWEIGHTS as free, so it
                # underestimates rec blocks and orders chain tails after the
                # next block's gate ACTs (head-of-line blocking).  Floors on
                # the chain ops pin the intended order; they sit below real
                # hardware pace, so they cost nothing at runtime.
                base = slk_base(SLK[0])
                SLK[0] += 1
                s = st[l]
                ps = ps_gates.tile([128, 16, BL], F32, tag="psg")
                def rec_mm(j):
                    for kc in range(KC):
                        nc.tensor.matmul(
                            ps[:, j, :],
                            whh_sb[:, kc, ts(j, 128)],
                            hist[:, kc, t - 1, :],
                            start=(j == 0 and kc == 0),
                            stop=False,
                            skip_group_check=True,
                        )
                if t > 0:
                    for j in range(12):
                        rec_mm(j)
                nc.tensor.matmul(
                    ps[:, 0:12, :],
                    ident,
                    ring[:, 0:12, ds(t * BL, BL)],
                    start=(t == 0),
                    stop=False,
                    skip_group_check=True,
                )
                if t > 0:
                    for j in range(12, 16):
                        rec_mm(j)
                nc.tensor.matmul(
                    ps[:, 12:16, :],
                    ident,
                    ring[:, 12:16, ds(t * BL, BL)],
                    start=False,
                    stop=True,
                    skip_group_check=True,
                )
                g = s["gates"]
                def chain_op(off, fn):
                    with tc.tile_wait_until(base + off), tc.high_priority(
                        offset=500000
                    ):
                        fn()
                chain_op(0.00220, lambda: nc.scalar.activation(
                    g[:, 0:12, :], ps[:, 0:12, :], func=AF.Sigmoid))
                chain_op(0.00230, lambda: nc.scalar.activation(
                    g[:, 12:16, :], ps[:, 12:16, :], func=AF.Tanh))
                if t == 0:
                    chain_op(0.00240, lambda: nc.vector.tensor_mul(
                        s["cT"], g[:, 0:4, :], g[:, 12:16, :]))
                else:
                    chain_op(0.00235, lambda: nc.vector.tensor_mul(
                        s["tmp2"], g[:, 4:8, :], s["cT"]))
                    chain_op(0.00240, lambda: nc.vector.tensor_mul(
                        s["tmp1"], g[:, 0:4, :], g[:, 12:16, :]))
                    chain_op(0.00245, lambda: nc.vector.tensor_add(
                        s["cT"], s["tmp1"], s["tmp2"]))
                chain_op(0.00255, lambda: nc.scalar.activation(
                    s["tanh_c"], s["cT"], func=AF.Tanh))
                chain_op(0.00265, lambda: nc.vector.tensor_mul(
                    hist[:, :, t, :], g[:, 8:12, :], s["tanh_c"]))

            # ---------------- fc helpers -----------------------------------
            fcw_tiles = {}
            fc_ct = [0]

            def fc_chunk_dma(ch, eng=None):
                v0 = ch * FCCH
                nvt = min(FCCH, VT - v0)
                fcw_sb = fcwp.tile(
                    [128, KC, FCCH * 128], E3M4, tag="fcw", name=f"fcw{ch}"
                )
                fcw_tiles[ch] = fcw_sb
                (eng or nc.gpsimd).dma_start(
                    out=fcw_sb[:, :, ds(0, nvt * 128)],
                    in_=fcw_d[:, :, ds(v0 * 128, nvt * 128)],
                )

            def fc_half(ch, half, alt_psum=False):
                v0 = ch * FCCH
                nvt = min(FCCH, VT - v0)
                fcw_sb = fcw_tiles[ch]
                if ch >= NCHUNKS - 2:
                    dma_engines = [nc.scalar, nc.sync]
                else:
                    dma_engines = [nc.sync, nc.gpsimd]
                for g0 in range(0, nvt, FCG):
                    ng = min(FCG, nvt - g0)
                    ot = fcstage.tile([128, FCG, 128], F32, tag="ot")
                    for j in range(ng):
                        vt = v0 + g0 + j
                        if alt_psum and (fc_ct[0] // 2) % 2 == 0:
                            ps = ps_gates.tile([128, 128], F32, tag="psg")
                        else:
                            ps = ps_wide.tile([128, 128], F32, tag="psw")
                        for kc in range(KC):
                            nc.tensor.matmul(
                                ps,
                                fcw_sb[:, kc, ts(g0 + j, 128)],
                                hist1[:, kc, ts(half, 16), :],
                                start=(kc == 0),
                                stop=(kc == KC - 1),
                            )
                        r = fc_ct[0] % 2
                        if r == 0:
                            nc.scalar.activation(
                                out=ot[:, j, :], in_=ps, func=AF.Identity,
                                bias=fcb_sb[:, vt : vt + 1], scale=1.0 / FCSCALE,
                            )
                        else:
                            nc.vector.tensor_scalar(
                                ot[:, j, :], ps, 1.0 / FCSCALE,
                                fcb_sb[:, vt : vt + 1], op0=ALU.mult, op1=ALU.add,
                            )
                        fc_ct[0] += 1
                    eng = dma_engines[(g0 // FCG) % 2]
                    eng.dma_start(
                        out=out_d[:, ds(v0 + g0, ng), ts(half, 128)],
                        in_=ot[:, ds(0, ng), :],
                    )

            with tc.tile_pool(name="wpool", bufs=1) as wpool:
                nc.scalar.dma_start(out=b0_sb, in_=b0_d[:])
                nc.scalar.dma_start(out=b1_sb, in_=b1_d[:])
                nc.scalar.dma_start(out=ident, in_=ident_d[:])
                nc.scalar.dma_start(out=fcb_sb, in_=fcb_d[:])

                xT_sb = wpool.tile([128, KC, NTOK], BF16, tag="xT")
                wih0_sb = wpool.tile([128, KC, G], BF16, tag="wih0")
                whh0_sb = wpool.tile([128, KC, G], BF16, tag="whh0")
                wih1_sb = wpool.tile([128, KC, G], BF16, tag="wih1")
                whh1_sb = wpool.tile([128, KC, G], BF16, tag="whh1")
                nc.sync.dma_start(
                    out=wih0_sb[:, :, ds(0, 1024)], in_=wih0_d[:, :, ds(0, 1024)]
                )
                nc.sync.dma_start(out=xT_sb, in_=xT_d[:])
                nc.sync.dma_start(
                    out=wih0_sb[:, :, ds(1024, 1024)],
                    in_=wih0_d[:, :, ds(1024, 1024)],
                )
                for piece in range(2):
                    nc.sync.dma_start(
                        out=whh0_sb[:, :, ts(piece, 1024)],
                        in_=whh0_d[:, :, ts(piece, 1024)],
                    )
                nc.sync.dma_start(out=wih1_sb, in_=wih1_d[:])
                nc.sync.dma_start(out=whh1_sb, in_=whh1_d[:])

                rec0 = dict(whh_sb=whh0_sb, ring=xp0r, hist=hist0)
                rec1 = dict(whh_sb=whh1_sb, ring=xp1r, hist=hist1)

                # prefetch the first 12 fc weight chunks BEHIND the LSTM
                # weights on the same FIFO queue: weights land first at full
                # DMA rate, then fcw streams during the DMA-idle LSTM phase
                for ch in range(12):
                    fc_chunk_dma(ch, eng=nc.sync)
                # first-layer input projection in token quarters: quarter 0
                # covers rec0 steps 0-7; the rest fill the rec0-solo gaps
                def xp0_quarter(q):
                    xp_block(
                        wih0_sb, lambda kc: xT_sb[:, kc, ds(q * 64, 64)],
                        b0_sb, xp0r, q * 64, 64, vec=True,
                    )

                xp0_quarter(0)
                for t in range(T):
                    rec_step(0, t, **rec0)
                    if t in (0, 2, 4):
                        xp0_quarter(t // 2 + 1)
                    if t % LAG == LAG - 1:
                        q = t // LAG
                        xp_block(
                            wih1_sb,
                            lambda kc: hist0[:, kc, ts(q, LAG), :],
                            b1_sb,
                            xp1r,
                            q * LAG * BL,
                            LAG * BL,
                            vec=True,
                        )
                    if t >= LAG:
                        rec_step(1, t - LAG, **rec1)
                # rec1 tail: fc h0 of the prefetched chunks fills the gaps
                for i, s_ in enumerate(range(T - LAG, T)):
                    rec_step(1, s_, **rec1)
                    fc_half(i, 0)

            # ================= fc remainder ================================
            # h1 of the resident chunks frees their buffers; stream the rest
            for ch in range(LAG, 12):
                fc_half(ch, 0, alt_psum=True)
            for ch in range(12):
                fc_half(ch, 1, alt_psum=True)
                if ch + 12 < NCHUNKS:
                    fc_chunk_dma(ch + 12, eng=nc.sync)
            for ch in range(12, NCHUNKS):
                fc_half(ch, 0, alt_psum=True)
                fc_half(ch, 1, alt_psum=True)
    return _patch_serialization(nc)


def _to_k128(W, dtype):
    """W [out_dim, K] -> [128, K//128, out_dim] with result[p,kc,g]=W[g,kc*128+p]."""
    K = W.shape[1]
    return np.ascontiguousarray(
        W.T.reshape(K // 128, 128, -1).transpose(1, 0, 2)
    ).astype(dtype)


# PyTorch gate order [i f g o] -> device order [i f o g]
_PERM = np.concatenate(
    [np.arange(0, 1024), np.arange(1536, 2048), np.arange(1024, 1536)]
)

_NC_CACHE = None
RUN_KWARGS = {}
LAST_RESULT = None


def kernel(
    sentence,
    features,
    lengths,
    emb,
    W_ih0,
    W_hh0,
    b_ih0,
    b_hh0,
    W_ih1,
    W_hh1,
    b_ih1,
    b_hh1,
    fc_W,
    fc_b,
):
    global _NC_CACHE, LAST_RESULT
    sentence = np.asarray(sentence).astype(np.int64)
    features = np.asarray(features, dtype=np.float32)
    emb = np.asarray(emb, dtype=np.float32)

    # embedding gather + teacher forcing shift (host; pure data movement)
    embeds = emb[sentence[:, : T - 1]]                      # [B, T-1, E]
    x = np.concatenate([features[:, None, :], embeds], axis=1)  # [B, T, E]

    wih0 = _to_k128(np.asarray(W_ih0, np.float32)[_PERM], BF16_NP)
    whh0 = _to_k128(np.asarray(W_hh0, np.float32)[_PERM], BF16_NP)
    wih1 = _to_k128(np.asarray(W_ih1, np.float32)[_PERM], BF16_NP)
    whh1 = _to_k128(np.asarray(W_hh1, np.float32)[_PERM], BF16_NP)
    b0 = np.ascontiguousarray(
        (np.asarray(b_ih0, np.float32) + np.asarray(b_hh0, np.float32))[_PERM]
        .reshape(16, 128)
        .T
    )
    b1 = np.ascontiguousarray(
        (np.asarray(b_ih1, np.float32) + np.asarray(b_hh1, np.float32))[_PERM]
        .reshape(16, 128)
        .T
    )
    fcw = _to_k128(
        np.asarray(fc_W, np.float32) * FCSCALE, E3M4_NP
    )                                                       # [128, KC, V]
    fcb = np.ascontiguousarray(
        np.asarray(fc_b, np.float32).reshape(VT, 128).T
    )

    common = {
        "wih0T": wih0,
        "whh0T": whh0,
        "wih1T": wih1,
        "whh1T": whh1,
        "b0": b0,
        "b1": b1,
        "ident": np.eye(128, dtype=BF16_NP),
        "fcwT": fcw,
        "fcb": fcb,
    }
    in_maps = []
    for c in range(NCORES):
        xc = x[c * BL : (c + 1) * BL]                       # [BL, T, E]
        # token-major [k, tok] with tok = t*BL + b
        xT = np.ascontiguousarray(xc.transpose(2, 1, 0).reshape(E, NTOK))
        xT_p = np.ascontiguousarray(
            xT.reshape(KC, 128, NTOK).transpose(1, 0, 2)
        ).astype(BF16_NP)
        in_maps.append({**common, "xT": xT_p})

    if _NC_CACHE is None:
        _NC_CACHE = _build_nc()

    res = run_bass_kernel_spmd(
        _NC_CACHE, in_maps, core_ids=list(range(NCORES)), **RUN_KWARGS
    )
    LAST_RESULT = res
    # per-core out: [128, VT, NTOK] (v = vt*128 + p) -> [V, T, BL] -> [BL, V, T]
    full = np.concatenate(
        [
            res.results[c]["out"]
            .transpose(1, 0, 2)
            .reshape(V, T, BL)
            .transpose(2, 0, 1)
            for c in range(NCORES)
        ],
        axis=0,
    )  # [B, V, T]
    return np.ascontiguousarray(full)


# revision 26
# speedup vs baseline: 1.0155x; 1.0044x over previous
"""Trainium2 Bass kernel for nn_Decoder (2-layer LSTM decoder + vocab head).

Computation (matches reference.py):
  embeds = emb[sentence]                      [B, T, E]
  x = concat(features, embeds[:, :-1])        [B, T, E]
  h0 = LSTM0(x), h1 = LSTM1(h0)               [B, T, H]
  out = (h1 @ fc_W.T + fc_b).transpose(0,2,1) [B, V, T]

Sharding (8 NeuronCores, SPMD, no collectives): data-parallel over batch.
Each core runs the full 2-layer LSTM for its 8 sequences and the full
32000-row vocab head for those sequences.  Compared to replicating the
LSTM and vocab-splitting the head, this cuts the per-core input
projections 8x at the cost of streaming the fc weight matrix per core
(e3m4, 16 MB), largely prefetched during the LSTM phase.

Layout: k-space (contraction dim on partitions), tokens t-major
(tok = t*8 + b).  Gates are reordered [i, f, o, g] on the host so the
three sigmoid gates are one contiguous ScalarE activation.  The per-core
output is [128, 250, 256] (p, vtile, tok); host maps v = vt*128 + p and
transposes to [B, V, T].

Schedule:
  xp0 half 0 -> rec0 starts; xp0 half 1 fills the first chain gap
  rec0(t), t in 0..31; after rec0 step 8q+7: xp1 quarter q (DVE move)
  rec1(t) lags by 8; during the rec1-only tail, fc chunks 0..7 token-half
  0 interleave as tensor filler; the rest of fc runs after.
fc weights are e3m4 (x256 scale, mixed-dtype matmul against bf16 h1);
the 1/256 compensation folds into the bias-add that reads PSUM.

Environment note: this walrus build rejects >1 embedded sync wait per
instruction; _split_waits_json() rewrites the serialized BIR, hoisting
excess waits onto same-engine NoOp carriers (identical semantics).
"""

import numpy as np
import ml_dtypes

import orjson
import concourse.tile as tile

_MAXW = 1


def _split_waits_json(b: bytes) -> bytes:
    d = orjson.loads(b)
    for f in d["functions"]:
        for blk in f["blocks"]:
            out = []
            for inst in blk["instructions"]:
                si = inst.get("sync_info")
                if si:
                    w = si.get("on_wait") or []
                    if len(w) > _MAXW:
                        for i, wt in enumerate(w[:-_MAXW]):
                            out.append(
                                {
                                    "debug": inst.get("debug", 0),
                                    "engine": inst["engine"],
                                    "ins": [],
                                    "outs": [],
                                    "name": f"{inst['name']}-hw{i}",
                                    "opcode": "NoOp",
                                    "sync_info": {"on_update": [], "on_wait": [wt]},
                                }
                            )
                        si["on_wait"] = w[-_MAXW:]
                out.append(inst)
            blk["instructions"] = out
    return orjson.dumps(d)


def _patch_serialization(nc):
    orig = nc.to_json_bytes
    nc.to_json_bytes = lambda: _split_waits_json(orig())
    return nc


import concourse.bass as bass
import concourse.mybir as mybir
from concourse.bass import ts, ds
from concourse.bass_utils import run_bass_kernel_spmd

F32 = mybir.dt.float32
BF16 = mybir.dt.bfloat16
E3M4 = mybir.dt.float8e3
AF = mybir.ActivationFunctionType
ALU = mybir.AluOpType
BF16_NP = ml_dtypes.bfloat16
E3M4_NP = ml_dtypes.float8_e3m4

E, H, V, B, T = 512, 512, 32000, 64, 32
G = 4 * H                    # 2048 gate rows per layer
KC = 4                       # 512 = 4 k-chunks of 128
NCORES = 8
BL = B // NCORES             # 8 sequences per core
NTOK = BL * T                # 256 tokens per core
VT = V // 128                # 250 vocab tiles
LAG = 8                      # rec1 runs LAG steps behind rec0
FCCH = 16                    # fc weight chunk: 16 vocab tiles
FCG = 4                      # vocab tiles staged per output DMA
FCSCALE = 256.0              # e3m4 weight scale (compensated at PSUM read)
NCHUNKS = (VT + FCCH - 1) // FCCH


def _build_nc():
    nc = bass.Bass()

    xT_d = nc.dram_tensor("xT", [128, KC, NTOK], BF16, kind="ExternalInput")
    wih0_d = nc.dram_tensor("wih0T", [128, KC, G], BF16, kind="ExternalInput")
    whh0_d = nc.dram_tensor("whh0T", [128, KC, G], BF16, kind="ExternalInput")
    wih1_d = nc.dram_tensor("wih1T", [128, KC, G], BF16, kind="ExternalInput")
    whh1_d = nc.dram_tensor("whh1T", [128, KC, G], BF16, kind="ExternalInput")
    b0_d = nc.dram_tensor("b0", [128, 16], F32, kind="ExternalInput")
    b1_d = nc.dram_tensor("b1", [128, 16], F32, kind="ExternalInput")
    ident_d = nc.dram_tensor("ident", [128, 128], BF16, kind="ExternalInput")
    fcw_d = nc.dram_tensor("fcwT", [128, KC, V], E3M4, kind="ExternalInput")
    fcb_d = nc.dram_tensor("fcb", [128, VT], F32, kind="ExternalInput")
    out_d = nc.dram_tensor("out", [128, VT, NTOK], F32, kind="ExternalOutput")

    with tile.TileContext(nc) as tc:
        with (
            tc.tile_pool(name="consts", bufs=1) as consts,
            tc.tile_pool(name="state", bufs=1) as state,
            tc.tile_pool(name="fcw", bufs=12) as fcwp,
            tc.tile_pool(name="fcstage", bufs=8) as fcstage,
            tc.tile_pool(name="ps_gates", bufs=4, space="PSUM") as ps_gates,
            tc.tile_pool(name="ps_wide", bufs=4, space="PSUM") as ps_wide,
        ):
            b0_sb = consts.tile([128, 16], F32, tag="b0")
            b1_sb = consts.tile([128, 16], F32, tag="b1")
            fcb_sb = consts.tile([128, VT], F32, tag="fcb")
            ident = consts.tile([128, 128], BF16, tag="ident")

            hist0 = consts.tile([128, KC, T, BL], BF16, tag="hist0")
            hist1 = consts.tile([128, KC, T, BL], BF16, tag="hist1")
            xp0r = consts.tile([128, 16, NTOK], BF16, tag="xp0r")
            xp1r = consts.tile([128, 16, NTOK], BF16, tag="xp1r")

            st = []
            for l in range(2):
                st.append(
                    dict(
                        cT=state.tile([128, KC, BL], F32, name=f"cT{l}"),
                        gates=state.tile([128, 16, BL], F32, name=f"gates{l}"),
                        tmp1=state.tile([128, KC, BL], F32, name=f"tmp1{l}"),
                        tmp2=state.tile([128, KC, BL], F32, name=f"tmp2{l}"),
                        tanh_c=state.tile([128, KC, BL], F32, name=f"tanhc{l}"),
                    )
                )

            def xp_block(w_sb, rhs, bias_sb, ring, n0, ncols, vec):
                """ring[:, g, n0:n0+ncols] = W.T @ rhs(kc) + bias, all 16 g."""
                for g in range(16):
                    ps = ps_wide.tile([128, ncols], F32, tag="psw")
                    for kc in range(KC):
                        nc.tensor.matmul(
                            ps,
                            w_sb[:, kc, ts(g, 128)],
                            rhs(kc),
                            start=(kc == 0),
                            stop=(kc == KC - 1),
                        )
                    if vec and g % 2 == 0:
                        nc.vector.tensor_scalar_add(
                            ring[:, g, ds(n0, ncols)], ps, bias_sb[:, g : g + 1]
                        )
                    else:
                        nc.scalar.activation(
                            out=ring[:, g, ds(n0, ncols)], in_=ps, func=AF.Identity,
                            bias=bias_sb[:, g : g + 1], scale=1.0,
                        )

            SLK = [0]                  # step-layer counter for chain pacing
            PACE = 0.0022              # ms per step-layer (both layers live)
            SOLO = 0.0042              # ms per step-layer while rec0 is alone
            NSOLO = 8
            RBASE = 0.008              # ms offset of the first rec block

            def slk_base(k):
                if k < NSOLO:
                    return RBASE + k * SOLO
                return RBASE + NSOLO * SOLO + (k - NSOLO) * PACE

            def rec_step(l, t, whh_sb, ring, hist):
                # gate order [i(0:4) f(4:8) o(8:12) g(12:16)].  The identity
                # fold is split so the sigmoid ACT (slabs 0:12) unblocks
                # before the tail of the matmul group finishes.
                # The scheduler's cost model treats LDWEIGHTS as free, so it
                # underestimates rec blocks and orders chain tails after the
                # next block's gate ACTs (head-of-line blocking).  Floors on
                # the chain ops pin the intended order; they sit below real
                # hardware pace, so they cost nothing at runtime.
                base = slk_base(SLK[0])
                SLK[0] += 1
                s = st[l]
                ps = ps_gates.tile([128, 16, BL], F32, tag="psg")
                def rec_mm(j):
                    for kc in range(KC):
                        nc.tensor.matmul(
                            ps[:, j, :],
                            whh_sb[:, kc, ts(j, 128)],
                            hist[:, kc, t - 1, :],
                            start=(j == 0 and kc == 0),
                            stop=False,
                            skip_group_check=True,
                        )
                if t > 0:
                    for j in range(12):
                        rec_mm(j)
                nc.tensor.matmul(
                    ps[:, 0:12, :],
                    ident,
                    ring[:, 0:12, ds(t * BL, BL)],
                    start=(t == 0),
                    stop=False,
                    skip_group_check=True,
                )
                if t > 0:
                    for j in range(12, 16):
                        rec_mm(j)
                nc.tensor.matmul(
                    ps[:, 12:16, :],
                    ident,
                    ring[:, 12:16, ds(t * BL, BL)],
                    start=False,
                    stop=True,
                    skip_group_check=True,
                )
                g = s["gates"]
                def chain_op(off, fn):
                    with tc.tile_wait_until(base + off), tc.high_priority(
                        offset=500000
                    ):
                        fn()
                chain_op(0.00220, lambda: nc.scalar.activation(
                    g[:, 0:12, :], ps[:, 0:12, :], func=AF.Sigmoid))
                chain_op(0.00230, lambda: nc.scalar.activation(
                    g[:, 12:16, :], ps[:, 12:16, :], func=AF.Tanh))
                if t == 0:
                    chain_op(0.00240, lambda: nc.vector.tensor_mul(
                        s["cT"], g[:, 0:4, :], g[:, 12:16, :]))
                else:
                    chain_op(0.00235, lambda: nc.vector.tensor_mul(
                        s["tmp2"], g[:, 4:8, :], s["cT"]))
                    chain_op(0.00240, lambda: nc.vector.tensor_mul(
                        s["tmp1"], g[:, 0:4, :], g[:, 12:16, :]))
                    chain_op(0.00245, lambda: nc.vector.tensor_add(
                        s["cT"], s["tmp1"], s["tmp2"]))
                chain_op(0.00255, lambda: nc.scalar.activation(
                    s["tanh_c"], s["cT"], func=AF.Tanh))
                chain_op(0.00265, lambda: nc.vector.tensor_mul(
                    hist[:, :, t, :], g[:, 8:12, :], s["tanh_c"]))

            # ---------------- fc helpers -----------------------------------
            fcw_tiles = {}
            fc_ct = [0]

            def fc_chunk_dma(ch, eng=None):
                v0 = ch * FCCH
                nvt = min(FCCH, VT - v0)
                fcw_sb = fcwp.tile(
                    [128, KC, FCCH * 128], E3M4, tag="fcw", name=f"fcw{ch}"
                )
                fcw_tiles[ch] = fcw_sb
                (eng or nc.gpsimd).dma_start(
                    out=fcw_sb[:, :, ds(0, nvt * 128)],
                    in_=fcw_d[:, :, ds(v0 * 128, nvt * 128)],
                )

            def fc_half(ch, half, alt_psum=False):
                v0 = ch * FCCH
                nvt = min(FCCH, VT - v0)
                fcw_sb = fcw_tiles[ch]
                if ch >= NCHUNKS - 4:
                    dma_engines = [nc.scalar, nc.sync]
                else:
                    dma_engines = [nc.sync, nc.gpsimd]
                for g0 in range(0, nvt, FCG):
                    ng = min(FCG, nvt - g0)
                    ot = fcstage.tile([128, FCG, 128], F32, tag="ot")
                    for j in range(ng):
                        vt = v0 + g0 + j
                        if alt_psum and (fc_ct[0] // 2) % 2 == 0:
                            ps = ps_gates.tile([128, 128], F32, tag="psg")
                        else:
                            ps = ps_wide.tile([128, 128], F32, tag="psw")
                        for kc in range(KC):
                            nc.tensor.matmul(
                                ps,
                                fcw_sb[:, kc, ts(g0 + j, 128)],
                                hist1[:, kc, ts(half, 16), :],
                                start=(kc == 0),
                                stop=(kc == KC - 1),
                            )
                        r = fc_ct[0] % 2
                        if r == 0:
                            nc.scalar.activation(
                                out=ot[:, j, :], in_=ps, func=AF.Identity,
                                bias=fcb_sb[:, vt : vt + 1], scale=1.0 / FCSCALE,
                            )
                        else:
                            nc.vector.tensor_scalar(
                                ot[:, j, :], ps, 1.0 / FCSCALE,
                                fcb_sb[:, vt : vt + 1], op0=ALU.mult, op1=ALU.add,
                            )
                        fc_ct[0] += 1
                    eng = dma_engines[(g0 // FCG) % 2]
                    eng.dma_start(
                        out=out_d[:, ds(v0 + g0, ng), ts(half, 128)],
                        in_=ot[:, ds(0, ng), :],
                    )

            with tc.tile_pool(name="wpool", bufs=1) as wpool:
                nc.scalar.dma_start(out=b0_sb, in_=b0_d[:])
                nc.scalar.dma_start(out=b1_sb, in_=b1_d[:])
                nc.scalar.dma_start(out=ident, in_=ident_d[:])
                nc.scalar.dma_start(out=fcb_sb, in_=fcb_d[:])

                xT_sb = wpool.tile([128, KC, NTOK], BF16, tag="xT")
                wih0_sb = wpool.tile([128, KC, G], BF16, tag="wih0")
                whh0_sb = wpool.tile([128, KC, G], BF16, tag="whh0")
                wih1_sb = wpool.tile([128, KC, G], BF16, tag="wih1")
                whh1_sb = wpool.tile([128, KC, G], BF16, tag="whh1")
                nc.sync.dma_start(
                    out=wih0_sb[:, :, ds(0, 1024)], in_=wih0_d[:, :, ds(0, 1024)]
                )
                nc.sync.dma_start(out=xT_sb, in_=xT_d[:])
                nc.sync.dma_start(
                    out=wih0_sb[:, :, ds(1024, 1024)],
                    in_=wih0_d[:, :, ds(1024, 1024)],
                )
                for piece in range(2):
                    nc.sync.dma_start(
                        out=whh0_sb[:, :, ts(piece, 1024)],
                        in_=whh0_d[:, :, ts(piece, 1024)],
                    )
                nc.sync.dma_start(out=wih1_sb, in_=wih1_d[:])
                nc.sync.dma_start(out=whh1_sb, in_=whh1_d[:])

                rec0 = dict(whh_sb=whh0_sb, ring=xp0r, hist=hist0)
                rec1 = dict(whh_sb=whh1_sb, ring=xp1r, hist=hist1)

                # prefetch the first 12 fc weight chunks BEHIND the LSTM
                # weights on the same FIFO queue: weights land first at full
                # DMA rate, then fcw streams during the DMA-idle LSTM phase
                for ch in range(12):
                    fc_chunk_dma(ch, eng=nc.sync)
                # first-layer input projection in token quarters: quarter 0
                # covers rec0 steps 0-7; the rest fill the rec0-solo gaps
                def xp0_quarter(q):
                    xp_block(
                        wih0_sb, lambda kc: xT_sb[:, kc, ds(q * 64, 64)],
                        b0_sb, xp0r, q * 64, 64, vec=True,
                    )

                xp0_quarter(0)
                for t in range(T):
                    rec_step(0, t, **rec0)
                    if t in (0, 2, 4):
                        xp0_quarter(t // 2 + 1)
                    if t % LAG == LAG - 1:
                        q = t // LAG
                        xp_block(
                            wih1_sb,
                            lambda kc: hist0[:, kc, ts(q, LAG), :],
                            b1_sb,
                            xp1r,
                            q * LAG * BL,
                            LAG * BL,
                            vec=True,
                        )
                    if t >= LAG:
                        rec_step(1, t - LAG, **rec1)
                # rec1 tail: fc h0 of the prefetched chunks fills the gaps
                for i, s_ in enumerate(range(T - LAG, T)):
                    rec_step(1, s_, **rec1)
                    fc_half(i, 0)

            # ================= fc remainder ================================
            # h1 of the resident chunks frees their buffers; stream the rest
            for ch in range(LAG, 12):
                fc_half(ch, 0, alt_psum=True)
            for ch in range(12):
                fc_half(ch, 1, alt_psum=True)
                if ch + 12 < NCHUNKS:
                    fc_chunk_dma(ch + 12, eng=nc.sync)
            for ch in range(12, NCHUNKS):
                fc_half(ch, 0, alt_psum=True)
                fc_half(ch, 1, alt_psum=True)
    return _patch_serialization(nc)


def _to_k128(W, dtype):
    """W [out_dim, K] -> [128, K//128, out_dim] with result[p,kc,g]=W[g,kc*128+p]."""
    K = W.shape[1]
    return np.ascontiguousarray(
        W.T.reshape(K // 128, 128, -1).transpose(1, 0, 2)
    ).astype(dtype)


# PyTorch gate order [i f g o] -> device order [i f o g]
_PERM = np.concatenate(
    [np.arange(0, 1024), np.arange(1536, 2048), np.arange(1024, 1536)]
)

_NC_CACHE = None
RUN_KWARGS = {}
LAST_RESULT = None


def kernel(
    sentence,
    features,
    lengths,
    emb,
    W_ih0,
    W_hh0,
    b_ih0,
    b_hh0,
    W_ih1,
    W_hh1,
    b_ih1,
    b_hh1,
    fc_W,
    fc_b,
):
    global _NC_CACHE, LAST_RESULT
    sentence = np.asarray(sentence).astype(np.int64)
    features = np.asarray(features, dtype=np.float32)
    emb = np.asarray(emb, dtype=np.float32)

    # embedding gather + teacher forcing shift (host; pure data movement)
    embeds = emb[sentence[:, : T - 1]]                      # [B, T-1, E]
    x = np.concatenate([features[:, None, :], embeds], axis=1)  # [B, T, E]

    wih0 = _to_k128(np.asarray(W_ih0, np.float32)[_PERM], BF16_NP)
    whh0 = _to_k128(np.asarray(W_hh0, np.float32)[_PERM], BF16_NP)
    wih1 = _to_k128(np.asarray(W_ih1, np.float32)[_PERM], BF16_NP)
    whh1 = _to_k128(np.asarray(W_hh1, np.float32)[_PERM], BF16_NP)
    b0 = np.ascontiguousarray(
        (np.asarray(b_ih0, np.float32) + np.asarray(b_hh0, np.float32))[_PERM]
        .reshape(16, 128)
        .T
    )
    b1 = np.ascontiguousarray(
        (np.asarray(b_ih1, np.float32) + np.asarray(b_hh1, np.float32))[_PERM]
        .reshape(16, 128)
        .T
    )
    fcw = _to_k128(
        np.asarray(fc_W, np.float32) * FCSCALE, E3M4_NP
    )                                                       # [128, KC, V]
    fcb = np.ascontiguousarray(
        np.asarray(fc_b, np.float32).reshape(VT, 128).T
    )

    common = {
        "wih0T": wih0,
        "whh0T": whh0,
        "wih1T": wih1,
        "whh1T": whh1,
        "b0": b0,
        "b1": b1,
        "ident": np.eye(128, dtype=BF16_NP),
        "fcwT": fcw,
        "fcb": fcb,
    }
    in_maps = []
    for c in range(NCORES):
        xc = x[c * BL : (c + 1) * BL]                       # [BL, T, E]
        # token-major [k, tok] with tok = t*BL + b
        xT = np.ascontiguousarray(xc.transpose(2, 1, 0).reshape(E, NTOK))
        xT_p = np.ascontiguousarray(
            xT.reshape(KC, 128, NTOK).transpose(1, 0, 2)
        ).astype(BF16_NP)
        in_maps.append({**common, "xT": xT_p})

    if _NC_CACHE is None:
        _NC_CACHE = _build_nc()

    res = run_bass_kernel_spmd(
        _NC_CACHE, in_maps, core_ids=list(range(NCORES)), **RUN_KWARGS
    )
    LAST_RESULT = res
    # per-core out: [128, VT, NTOK] (v = vt*128 + p) -> [V, T, BL] -> [BL, V, T]
    full = np.concatenate(
        [
            res.results[c]["out"]
            .transpose(1, 0, 2)
            .reshape(V, T, BL)
            .transpose(2, 0, 1)
            for c in range(NCORES)
        ],
        axis=0,
    )  # [B, V, T]
    return np.ascontiguousarray(full)
